# revision 2
# baseline (speedup 1.0000x reference)
"""Trainium2 Bass kernel for nn_CAM_Module (channel-attention module).

Math per batch n (N = B*D = 128 independent problems):
    V = x[b, :, d, :, :].reshape(C, S)          # C=128, S=4096
    G = V @ V.T                                  # (C, C) Gram / energy
    A = softmax(-G) row-wise (stabilized with rowmin subtract)
    out_n = (gamma * A + I) @ V                  # == gamma*(A@V) + V

Sharding: data-parallel over n across 8 NeuronCores (16 n per core).

v2 design (vs the fp32 baseline at ~197us):
  - fp16 I/O: x is cast to fp16 on the host (free) and the output is
    written back as fp16 (rel-err ~2^-11, far under the 2e-2 gate).
    Halves HBM traffic: 33.6 MB/core -> ~94 us DMA floor.
  - V^T via the XBAR DMA-transpose (InstDmaTransposeAnt, 14 ns per
    16x128 tile) SBUF->SBUF on the ACT HWDGE ring: no PE transposes,
    no PSUM->SBUF copy pass.  PE only runs the two real GEMMs
    (32+8 matmuls/batch at fp16 FWL rate) + one 128x128 softmax
    transpose: ~3.8 us/batch -> ~61 us/core, under the DMA floor.
  - Engine layout per batch: sync ring = input loads; ACT ring =
    V^T transposes; ACT = exp; DVE = rowmin/recip/epilogue;
    gpsimd/SWDGE = output stores.
"""

import numpy as np
from contextlib import ExitStack

import concourse.bass as bass
import concourse.tile as tile
from concourse import bacc, mybir
from concourse.bass_utils import run_bass_kernel_spmd

B, C, D, H, W = 4, 128, 32, 64, 64
S = H * W                  # 4096
N_TOTAL = B * D            # 128
N_CORES = 8
N_PER_CORE = N_TOTAL // N_CORES   # 16

FP = mybir.dt.float32
FP16 = mybir.dt.float16
AF = mybir.ActivationFunctionType
AX = mybir.AxisListType
OP = mybir.AluOpType

_CACHE = {}


def build_program(n_per_core=N_PER_CORE):
    key = n_per_core
    if key in _CACHE:
        return _CACHE[key]

    nc = bacc.Bacc(
        "TRN2", target_bir_lowering=False, debug=False, num_devices=N_CORES
    )
    xs = nc.dram_tensor("xs", [n_per_core, C, S], FP16, kind="ExternalInput").ap()
    gamma_b = nc.dram_tensor("gamma_b", [C, 1], FP, kind="ExternalInput").ap()
    ident = nc.dram_tensor("ident", [C, C], FP, kind="ExternalInput").ap()
    out = nc.dram_tensor("out", [n_per_core, C, S], FP16, kind="ExternalOutput").ap()

    NCHUNK = S // C            # 32 Gram chunks per n
    NJ = S // 512              # 8 512-wide column groups for stage 2
    PIPE = 2                   # stage-2 depth: st2(n) emitted in iter n+PIPE
    LOOK = 3                   # V-load lookahead: iter n issues v(n+LOOK) DMAs
    TRLOOK = 2                 # V^T lookahead: iter n issues xbar(n+TRLOOK)

    with tile.TileContext(nc) as tc, ExitStack() as ctx:
        const_pool = ctx.enter_context(tc.tile_pool(name="const", bufs=1))
        v_pool = ctx.enter_context(tc.tile_pool(name="v", bufs=LOOK + PIPE + 1))
        u_pool = ctx.enter_context(tc.tile_pool(name="u", bufs=TRLOOK + 2))
        small_pool = ctx.enter_context(tc.tile_pool(name="small", bufs=PIPE + 2))
        osb_pool = ctx.enter_context(tc.tile_pool(name="osb", bufs=3))
        g_ps_pool = ctx.enter_context(tc.tile_pool(name="gps", bufs=2, space="PSUM"))
        at_ps_pool = ctx.enter_context(tc.tile_pool(name="atps", bufs=2, space="PSUM"))
        o_ps_pool = ctx.enter_context(tc.tile_pool(name="ops", bufs=4, space="PSUM"))

        id_sb = const_pool.tile([C, C], FP)
        nc.sync.dma_start(id_sb[:], ident[:])
        gam_sb = const_pool.tile([C, 1], FP)
        nc.sync.dma_start(gam_sb[:], gamma_b[:])

        pend = []  # [(n, v_sb, abt, gz), ...] awaiting stage 2

        def emit_stage2():
            # out_n = gz_c * (numer^T-matmul) + V  with gz = gamma/Z
            pn, pv_sb, abt, gz = pend.pop(0)
            o_sb = osb_pool.tile([C, S], FP16, tag="o_sb")
            for j in range(NJ):
                o_ps = o_ps_pool.tile([C, 512], FP, tag="o_ps")
                nc.tensor.matmul(
                    o_ps[:],
                    abt[:],
                    pv_sb[:, 512 * j : 512 * (j + 1)],
                    start=True, stop=True,
                )
                # fused epilogue: scale by gamma/Z and add V (the residual y)
                nc.vector.scalar_tensor_tensor(
                    o_sb[:, 512 * j : 512 * (j + 1)],
                    o_ps[:],
                    gz[:],
                    pv_sb[:, 512 * j : 512 * (j + 1)],
                    op0=OP.mult, op1=OP.add,
                )
                if j % 4 == 3:
                    # outputs leave in halves via GpSimd/SWDGE: they never
                    # block input/transpose issue on the HWDGE rings
                    h = j // 4
                    nc.gpsimd.dma_start(
                        out[pn, :, 2048 * h : 2048 * (h + 1)],
                        o_sb[:, 2048 * h : 2048 * (h + 1)],
                    )

        vmap = {}
        umap = {}

        def load_v(m):
            if not (0 <= m < n_per_core) or m in vmap:
                return
            t = v_pool.tile([C, S], FP16, tag="v_sb")
            for h in range(2):
                nc.sync.dma_start(
                    t[:, 2048 * h : 2048 * (h + 1)],
                    xs[m, :, 2048 * h : 2048 * (h + 1)],
                )
            vmap[m] = t

        def tr_v(m):
            # u[p, k, c] = v[c, 128k + p]: exact V^T chunks via the DMA
            # XBAR transpose, issued on the ACT HWDGE ring (sync ring is
            # busy with input loads).  Two halves so each can start as
            # soon as its input-load DMA lands.
            if not (0 <= m < n_per_core) or m in umap:
                return
            v_sb = vmap[m]
            u = u_pool.tile([C, NCHUNK, C], FP16, tag="u_sb")
            for h in range(2):
                nc.scalar.dma_start(
                    u[:, 16 * h : 16 * (h + 1), :],
                    v_sb[:, 2048 * h : 2048 * (h + 1)],
                    transpose=True,
                )
            umap[m] = u

        for m in range(LOOK):
            load_v(m)
        for m in range(TRLOOK):
            tr_v(m)

        for n in range(n_per_core + PIPE):
            if n < n_per_core:
                load_v(n + LOOK)
                tr_v(n + TRLOOK)
                v_sb = vmap.pop(n)
                u_sb = umap.pop(n)
                g_ps = g_ps_pool.tile([C, C], FP, tag="g_ps")

                def mm1(k0, k1):
                    for k in range(k0, k1):
                        ck = u_sb[:, k, :]
                        nc.tensor.matmul(
                            g_ps[:], ck, ck,
                            start=(k == 0), stop=(k == NCHUNK - 1),
                        )

                mm1(0, NCHUNK // 2)
                if len(pend) >= PIPE:
                    emit_stage2()
                mm1(NCHUNK // 2, NCHUNK)

                # softmax: critical chain is only rmin -> exp -> transpose;
                # normalization (recip, *gamma) runs off-path, applied in the
                # stage-2 epilogue.
                rmin = small_pool.tile([C, 1], FP, tag="rmin")
                nc.vector.tensor_reduce(rmin[:], g_ps[:], axis=AX.X, op=OP.min)
                numer = small_pool.tile([C, C], FP, tag="numer")
                zsum = small_pool.tile([C, 1], FP, tag="zsum")
                nc.scalar.activation(
                    numer[:], g_ps[:], AF.Exp,
                    bias=rmin[:], scale=-1.0, accum_out=zsum[:],
                )
                at_ps = at_ps_pool.tile([C, C], FP, tag="at_ps")
                nc.tensor.transpose(at_ps[:], numer[:], id_sb[:])
                abt = small_pool.tile([C, C], FP16, tag="abt")
                nc.vector.tensor_copy(abt[:], at_ps[:])
                zinv = small_pool.tile([C, 1], FP, tag="zinv")
                nc.vector.reciprocal(zinv[:], zsum[:])
                gz = small_pool.tile([C, 1], FP, tag="gz")
                nc.vector.tensor_mul(gz[:], zinv[:], gam_sb[:])
                pend.append((n, v_sb, abt, gz))
            else:
                if pend:
                    emit_stage2()

    nc.compile()
    _CACHE[key] = nc
    return nc


def make_in_maps(x, gamma, n_per_core=N_PER_CORE):
    """Shard full inputs into per-core input maps (data-parallel over B*D)."""
    x = np.asarray(x, dtype=np.float32)
    gamma = np.asarray(gamma, dtype=np.float32).reshape(-1)
    gamma_b = np.full((C, 1), gamma[0], dtype=np.float32)
    ident = np.eye(C, dtype=np.float32)
    # v[n=(b,d)][c,s] = x[b,c,d,s] ; core i takes n in [i*npc, (i+1)*npc)
    xt = np.ascontiguousarray(
        x.reshape(B, C, D, S).transpose(0, 2, 1, 3)
    ).reshape(N_TOTAL, C, S).astype(np.float16)
    in_maps = []
    for i in range(N_CORES):
        xs = np.ascontiguousarray(xt[i * n_per_core : (i + 1) * n_per_core])
        in_maps.append({"xs": xs, "gamma_b": gamma_b, "ident": ident})
    return in_maps


def run_on_cores(x, gamma, trace=False, **kw):
    nc = build_program()
    in_maps = make_in_maps(x, gamma)
    res = run_bass_kernel_spmd(
        nc, in_maps, core_ids=list(range(N_CORES)), trace=trace, **kw
    )
    return res


def assemble_output(results):
    parts = [results[i]["out"] for i in range(N_CORES)]
    full = np.concatenate(parts, axis=0).astype(np.float32)  # (B*D, C, S)
    # reference returns a raw reinterpret of contiguous (B, D, C, H, W)
    return full.reshape(B, C, D, H, W)


def kernel(x, gamma):
    res = run_on_cores(x, gamma, trace=False)
    return assemble_output(res.results)


# revision 7
# speedup vs baseline: 1.2057x; 1.2057x over previous
"""Trainium2 Bass kernel for nn_CAM_Module (channel-attention module).

Math per batch n (N = B*D = 128 independent problems):
    V = x[b, :, d, :, :].reshape(C, S)          # C=128, S=4096
    G = V @ V.T                                  # (C, C) Gram / energy
    A = softmax(-G) row-wise (stabilized with rowmin subtract)
    out_n = (gamma * A + I) @ V                  # == gamma*(A@V) + V

Sharding: data-parallel over n across 8 NeuronCores (16 n per core).

v3 design (fp32 baseline ~197us, XBAR-transpose attempt 306us):
  - fp16 I/O: x cast to fp16 on the host, output written as fp16
    (rel-err ~2^-11, far under the 2e-2 gate).  Halves HBM traffic;
    the 16 shared DMA engines are the binding resource (~100us for
    33.6 MB of in+out).  The XBAR DMA-transpose was abandoned: its
    256B packets cost ~90us of DMA-engine time, serializing with I/O.
  - V^T built on the PE with pair-packed transposes: fp16 pairs
    bitcast to int32 ride the 4-byte transpose fast path (4x faster
    than 16-bit transpose mode), 16 instead of 32 transposes/batch,
    bit-exact (int32 datapath, no denormal flush).  The transposed
    words interleave the two fp16 parities along the free axis; a
    single strided copy per PSUM tile (DVE/ACT, 1x mode) de-interleaves
    into contiguous 128-wide V^T chunks.  The induced permutation of
    the s-axis is harmless: s only appears as the Gram contraction
    index.
  - Residual folded into the stage-2 matrix: abt = (gamma/Z * A)^T + I
    built once per batch (3 small 128x128 ops), so the stage-2 matmul
    o = abt.T @ V = gamma*A@V + V emits the FINAL values into PSUM and
    the epilogue is a plain PSUM->SBUF fp16 copy (no STT, no add).
  - Engine layout per batch: PE 16 tr + 32 Gram mm + 1 softmax tr +
    8 stage-2 mm (~5.4us); DVE/ACT split the de-interleave + epilogue
    copies + softmax chain (~4.5us each); sync ring = input loads;
    gpsimd/SWDGE = output stores.
"""

import numpy as np
from contextlib import ExitStack

import concourse.bass as bass
import concourse.tile as tile
from concourse import bacc, mybir
from concourse.bass_utils import run_bass_kernel_spmd

B, C, D, H, W = 4, 128, 32, 64, 64
S = H * W                  # 4096
N_TOTAL = B * D            # 128
N_CORES = 8
N_PER_CORE = N_TOTAL // N_CORES   # 16

FP = mybir.dt.float32
FP16 = mybir.dt.float16
I32 = mybir.dt.int32
AF = mybir.ActivationFunctionType
AX = mybir.AxisListType
OP = mybir.AluOpType

_CACHE = {}

TR_DT = FP    # packed-transpose dtype (fp32 4-byte transpose fast path)


def build_program(n_per_core=N_PER_CORE):
    key = n_per_core
    if key in _CACHE:
        return _CACHE[key]

    nc = bacc.Bacc(
        "TRN2", target_bir_lowering=False, debug=False, num_devices=N_CORES
    )
    xs = nc.dram_tensor("xs", [n_per_core, C, S], FP16, kind="ExternalInput").ap()
    gamma_b = nc.dram_tensor("gamma_b", [C, 1], FP, kind="ExternalInput").ap()
    ident = nc.dram_tensor("ident", [C, C], FP, kind="ExternalInput").ap()
    out = nc.dram_tensor("out", [n_per_core, C, S], FP16, kind="ExternalOutput").ap()

    NCHUNK = S // C            # 32 V^T chunks per n
    NW = S // 2                # 2048 packed int32 words per partition
    NT = 4                     # PSUM transpose tiles per batch (4 tr each)
    NJ = S // 512              # 8 512-wide column groups for stage 2
    PIPE = 2                   # stage-2 depth: st2(n) emitted in iter n+PIPE
    LOOK = 4                   # V-load lookahead
    TRLOOK = 2                 # V^T lookahead

    with tile.TileContext(nc) as tc, ExitStack() as ctx:
        const_pool = ctx.enter_context(tc.tile_pool(name="const", bufs=1))
        v_pool = ctx.enter_context(tc.tile_pool(name="v", bufs=LOOK + PIPE + 1))
        u_pool = ctx.enter_context(tc.tile_pool(name="u", bufs=TRLOOK + 2))
        small_pool = ctx.enter_context(tc.tile_pool(name="small", bufs=PIPE + 2))
        osb_pool = ctx.enter_context(tc.tile_pool(name="osb", bufs=3))
        tr_ps_pool = ctx.enter_context(tc.tile_pool(name="trps", bufs=2, space="PSUM"))
        g_ps_pool = ctx.enter_context(tc.tile_pool(name="gps", bufs=2, space="PSUM"))
        at_ps_pool = ctx.enter_context(tc.tile_pool(name="atps", bufs=1, space="PSUM"))
        o_ps_pool = ctx.enter_context(tc.tile_pool(name="ops", bufs=3, space="PSUM"))

        id_sb = const_pool.tile([C, C], FP)
        nc.sync.dma_start(id_sb[:], ident[:])
        gam_sb = const_pool.tile([C, 1], FP)
        nc.sync.dma_start(gam_sb[:], gamma_b[:])

        pend = []  # [(n, v_sb, abt), ...] awaiting stage 2

        def emit_stage2():
            # o = abt.T @ V = gz*A@V + V: final values straight from PSUM
            pn, pv_sb, abt = pend.pop(0)
            o_sb = osb_pool.tile([C, S], FP16, tag="o_sb")
            for j in range(NJ):
                o_ps = o_ps_pool.tile([C, 512], FP, tag="o_ps")
                nc.tensor.matmul(
                    o_ps[:],
                    abt[:],
                    pv_sb[:, 512 * j : 512 * (j + 1)],
                    start=True, stop=True,
                )
                if j % 2 == 0:
                    nc.vector.tensor_copy(o_sb[:, 512 * j : 512 * (j + 1)], o_ps[:])
                else:
                    nc.scalar.copy(o_sb[:, 512 * j : 512 * (j + 1)], o_ps[:])
                if j % 4 == 3:
                    h = j // 4
                    nc.gpsimd.dma_start(
                        out[pn, :, 2048 * h : 2048 * (h + 1)],
                        o_sb[:, 2048 * h : 2048 * (h + 1)],
                    )

        vmap = {}
        umap = {}

        def load_v(m):
            if not (0 <= m < n_per_core) or m in vmap:
                return
            t = v_pool.tile([C, S], FP16, tag="v_sb")
            for h in range(2):
                nc.sync.dma_start(
                    t[:, 2048 * h : 2048 * (h + 1)],
                    xs[m, :, 2048 * h : 2048 * (h + 1)],
                )
            vmap[m] = t

        def tr_v(m):
            # V^T via pair-packed int32 PE transposes + strided de-interleave.
            # t_ps[p, 128q + w] (int32) = vp[w-th c?..]: transpose of packed
            # words; fp16 view row p of tile t: [c0_even c0_odd c1_even ...]
            # for s-pair p of packed chunk 4t+q.  The copy splits parities to
            # u chunks (8t + 2q + parity), each an exact 128x128 V^T chunk
            # (s-order permuted, which the Gram contraction doesn't care
            # about).
            if not (0 <= m < n_per_core) or m in umap:
                return
            v_sb = vmap[m]
            vp = v_sb.bitcast(TR_DT)          # (C, NW) packed words
            idt = id_sb.bitcast(TR_DT)
            u = u_pool.tile([C, NCHUNK, C], FP16, tag="u_sb")
            for t in range(NT):
                t_ps = tr_ps_pool.tile([C, 512], TR_DT, tag="t_ps")
                for q in range(4):
                    k = 4 * t + q
                    nc.tensor.transpose(
                        t_ps[:, 128 * q : 128 * (q + 1)],
                        vp[:, 128 * k : 128 * (k + 1)],
                        idt,
                    )
                # de-interleave: contiguous fp16 read of the PSUM tile,
                # strided write into 8 contiguous V^T chunks (1x mode).
                src = t_ps.bitcast(FP16)      # (C, 1024): q(4) x c(128) x par(2)
                src3 = src.rearrange("p (q c r) -> p q c r", q=4, c=C)
                dst3 = u[:, 8 * t : 8 * t + 8, :].rearrange(
                    "p (q r) c -> p q c r", q=4
                )
                if t % 2 == 0:
                    nc.vector.tensor_copy(dst3, src3)
                else:
                    nc.scalar.copy(dst3, src3)
            umap[m] = u

        for m in range(LOOK):
            load_v(m)
        for m in range(TRLOOK):
            tr_v(m)

        for n in range(n_per_core + PIPE):
            if n < n_per_core:
                load_v(n + LOOK)
                v_sb = vmap.pop(n)
                u_sb = umap.pop(n)
                g_ps = g_ps_pool.tile([C, C], FP, tag="g_ps")

                def mm1(k0, k1):
                    for k in range(k0, k1):
                        ck = u_sb[:, k, :]
                        nc.tensor.matmul(
                            g_ps[:], ck, ck,
                            start=(k == 0), stop=(k == NCHUNK - 1),
                        )

                mm1(0, NCHUNK // 2)
                tr_v(n + TRLOOK)
                if len(pend) >= PIPE:
                    emit_stage2()
                mm1(NCHUNK // 2, NCHUNK)

                # softmax: rmin -> exp (with accumulated Z) -> fold gamma/Z
                # into numer -> transpose -> add I: abt = (gz*A)^T + I
                rmin = small_pool.tile([C, 1], FP, tag="rmin")
                nc.vector.tensor_reduce(rmin[:], g_ps[:], axis=AX.X, op=OP.min)
                numer = small_pool.tile([C, C], FP, tag="numer")
                zsum = small_pool.tile([C, 1], FP, tag="zsum")
                nc.scalar.activation(
                    numer[:], g_ps[:], AF.Exp,
                    bias=rmin[:], scale=-1.0, accum_out=zsum[:],
                )
                zinv = small_pool.tile([C, 1], FP, tag="zinv")
                nc.vector.reciprocal(zinv[:], zsum[:])
                gz = small_pool.tile([C, 1], FP, tag="gz")
                nc.vector.tensor_mul(gz[:], zinv[:], gam_sb[:])
                numer_s = small_pool.tile([C, C], FP, tag="numer_s")
                nc.vector.tensor_scalar(numer_s[:], numer[:], gz[:], None, OP.mult)
                at_ps = at_ps_pool.tile([C, C], FP, tag="at_ps")
                nc.tensor.transpose(at_ps[:], numer_s[:], id_sb[:])
                abt = small_pool.tile([C, C], FP16, tag="abt")
                nc.vector.tensor_add(abt[:], at_ps[:], id_sb[:])
                pend.append((n, v_sb, abt))
            else:
                if pend:
                    emit_stage2()

    nc.compile()
    _CACHE[key] = nc
    return nc


def make_in_maps(x, gamma, n_per_core=N_PER_CORE):
    """Shard full inputs into per-core input maps (data-parallel over B*D)."""
    x = np.asarray(x, dtype=np.float32)
    gamma = np.asarray(gamma, dtype=np.float32).reshape(-1)
    gamma_b = np.full((C, 1), gamma[0], dtype=np.float32)
    ident = np.eye(C, dtype=np.float32)
    # v[n=(b,d)][c,s] = x[b,c,d,s] ; core i takes n in [i*npc, (i+1)*npc)
    xt = np.ascontiguousarray(
        x.reshape(B, C, D, S).transpose(0, 2, 1, 3)
    ).reshape(N_TOTAL, C, S).astype(np.float16)
    in_maps = []
    for i in range(N_CORES):
        xs = np.ascontiguousarray(xt[i * n_per_core : (i + 1) * n_per_core])
        in_maps.append({"xs": xs, "gamma_b": gamma_b, "ident": ident})
    return in_maps


def run_on_cores(x, gamma, trace=False, **kw):
    nc = build_program()
    in_maps = make_in_maps(x, gamma)
    res = run_bass_kernel_spmd(
        nc, in_maps, core_ids=list(range(N_CORES)), trace=trace, **kw
    )
    return res


def assemble_output(results):
    parts = [results[i]["out"] for i in range(N_CORES)]
    full = np.concatenate(parts, axis=0).astype(np.float32)  # (B*D, C, S)
    # reference returns a raw reinterpret of contiguous (B, D, C, H, W)
    return full.reshape(B, C, D, H, W)


def kernel(x, gamma):
    res = run_on_cores(x, gamma, trace=False)
    return assemble_output(res.results)


# revision 12
# speedup vs baseline: 2.4629x; 2.0427x over previous
"""Trainium2 Bass kernel for nn_CAM_Module (channel-attention module).

Math per batch n (N = B*D = 128 independent problems):
    V = x[b, :, d, :, :].reshape(C, S)          # C=128, S=4096
    G = V @ V.T                                  # (C, C) Gram / energy
    A = softmax(-G) row-wise (stabilized with rowmin subtract)
    out_n = (gamma * A + I) @ V                  # == gamma*(A@V) + V

Sharding: data-parallel over n across 8 NeuronCores (16 n per core).

v3 design (fp32 baseline ~197us, XBAR-transpose attempt 306us):
  - fp16 I/O: x cast to fp16 on the host, output written as fp16
    (rel-err ~2^-11, far under the 2e-2 gate).  Halves HBM traffic;
    the 16 shared DMA engines are the binding resource (~100us for
    33.6 MB of in+out).  The XBAR DMA-transpose was abandoned: its
    256B packets cost ~90us of DMA-engine time, serializing with I/O.
  - V^T built on the PE with pair-packed transposes: fp16 pairs
    bitcast to int32 ride the 4-byte transpose fast path (4x faster
    than 16-bit transpose mode), 16 instead of 32 transposes/batch,
    bit-exact (int32 datapath, no denormal flush).  The transposed
    words interleave the two fp16 parities along the free axis; a
    single strided copy per PSUM tile (DVE/ACT, 1x mode) de-interleaves
    into contiguous 128-wide V^T chunks.  The induced permutation of
    the s-axis is harmless: s only appears as the Gram contraction
    index.
  - Residual folded into the stage-2 matrix: abt = (gamma/Z * A)^T + I
    built once per batch (3 small 128x128 ops), so the stage-2 matmul
    o = abt.T @ V = gamma*A@V + V emits the FINAL values into PSUM and
    the epilogue is a plain PSUM->SBUF fp16 copy (no STT, no add).
  - Engine layout per batch: PE 16 tr + 32 Gram mm + 1 softmax tr +
    8 stage-2 mm (~5.4us); DVE/ACT split the de-interleave + epilogue
    copies + softmax chain (~4.5us each); sync ring = input loads;
    gpsimd/SWDGE = output stores.
"""

import numpy as np
from contextlib import ExitStack

import concourse.bass as bass
import concourse.tile as tile
from concourse import bacc, mybir
from concourse.bass_utils import run_bass_kernel_spmd

B, C, D, H, W = 4, 128, 32, 64, 64
S = H * W                  # 4096
N_TOTAL = B * D            # 128
N_CORES = 8
N_PER_CORE = N_TOTAL // N_CORES   # 16

FP = mybir.dt.float32
FP16 = mybir.dt.float16
I32 = mybir.dt.int32
AF = mybir.ActivationFunctionType
AX = mybir.AxisListType
OP = mybir.AluOpType

_CACHE = {}

TR_DT = FP    # packed-transpose dtype (fp32 4-byte transpose fast path)


def build_program(n_per_core=N_PER_CORE):
    key = n_per_core
    if key in _CACHE:
        return _CACHE[key]

    nc = bacc.Bacc(
        "TRN2", target_bir_lowering=False, debug=False, num_devices=N_CORES
    )
    xs = nc.dram_tensor("xs", [n_per_core, C, S], FP16, kind="ExternalInput").ap()
    gamma_b = nc.dram_tensor("gamma_b", [C, 1], FP, kind="ExternalInput").ap()
    ident = nc.dram_tensor("ident", [C, C], FP, kind="ExternalInput").ap()
    out = nc.dram_tensor("out", [n_per_core, C, S], FP16, kind="ExternalOutput").ap()

    NCHUNK = S // C            # 32 V^T chunks per n
    NW = S // 2                # 2048 packed int32 words per partition
    NT = 4                     # PSUM transpose tiles per batch (4 tr each)
    NJ = S // 512              # 8 512-wide column groups for stage 2
    PIPE = 2                   # stage-2 depth: st2(n) emitted in iter n+PIPE
    LOOK = 4                   # V-load lookahead
    TRLOOK = 2                 # V^T lookahead

    with tile.TileContext(nc) as tc, ExitStack() as ctx:
        const_pool = ctx.enter_context(tc.tile_pool(name="const", bufs=1))
        v_pool = ctx.enter_context(tc.tile_pool(name="v", bufs=LOOK + PIPE + 1))
        u_pool = ctx.enter_context(tc.tile_pool(name="u", bufs=TRLOOK + 2))
        small_pool = ctx.enter_context(tc.tile_pool(name="small", bufs=PIPE + 2))
        osb_pool = ctx.enter_context(tc.tile_pool(name="osb", bufs=3))
        tr_ps_pool = ctx.enter_context(tc.tile_pool(name="trps", bufs=2, space="PSUM"))
        g_ps_pool = ctx.enter_context(tc.tile_pool(name="gps", bufs=2, space="PSUM"))
        at_ps_pool = ctx.enter_context(tc.tile_pool(name="atps", bufs=1, space="PSUM"))
        o_ps_pool = ctx.enter_context(tc.tile_pool(name="ops", bufs=3, space="PSUM"))

        id_sb = const_pool.tile([C, C], FP)
        nc.sync.dma_start(id_sb[:], ident[:])
        gam_sb = const_pool.tile([C, 1], FP)
        nc.sync.dma_start(gam_sb[:], gamma_b[:])

        pend = []  # [(n, v_sb, numer_s), ...] awaiting stage 2

        def emit_stage2():
            # abt = (gz*A)^T + I built here, PIPE iterations after the
            # softmax chain produced numer_s, so the PE never waits on it.
            # o = abt.T @ V = gz*A@V + V: final values straight from PSUM.
            pn, pv_sb, numer_s = pend.pop(0)
            at_ps = at_ps_pool.tile([C, C], FP, tag="at_ps")
            nc.tensor.transpose(at_ps[:], numer_s[:], id_sb[:])
            abt = small_pool.tile([C, C], FP16, tag="abt")
            nc.vector.tensor_add(abt[:], at_ps[:], id_sb[:])
            o_sb = osb_pool.tile([C, S], FP16, tag="o_sb")
            for j in range(NJ):
                o_ps = o_ps_pool.tile([C, 512], FP, tag="o_ps")
                nc.tensor.matmul(
                    o_ps[:],
                    abt[:],
                    pv_sb[:, 512 * j : 512 * (j + 1)],
                    start=True, stop=True,
                )
                if j % 2 == 0:
                    nc.vector.tensor_copy(o_sb[:, 512 * j : 512 * (j + 1)], o_ps[:])
                else:
                    nc.scalar.copy(o_sb[:, 512 * j : 512 * (j + 1)], o_ps[:])
                if j % 4 == 3:
                    h = j // 4
                    nc.gpsimd.dma_start(
                        out[pn, :, 2048 * h : 2048 * (h + 1)],
                        o_sb[:, 2048 * h : 2048 * (h + 1)],
                    )

        vmap = {}
        umap = {}

        def load_v(m):
            if not (0 <= m < n_per_core) or m in vmap:
                return
            t = v_pool.tile([C, S], FP16, tag="v_sb")
            for h in range(2):
                nc.sync.dma_start(
                    t[:, 2048 * h : 2048 * (h + 1)],
                    xs[m, :, 2048 * h : 2048 * (h + 1)],
                )
            vmap[m] = t

        def tr_v(m):
            # V^T via pair-packed int32 PE transposes + strided de-interleave.
            # t_ps[p, 128q + w] (int32) = vp[w-th c?..]: transpose of packed
            # words; fp16 view row p of tile t: [c0_even c0_odd c1_even ...]
            # for s-pair p of packed chunk 4t+q.  The copy splits parities to
            # u chunks (8t + 2q + parity), each an exact 128x128 V^T chunk
            # (s-order permuted, which the Gram contraction doesn't care
            # about).
            if not (0 <= m < n_per_core) or m in umap:
                return
            v_sb = vmap[m]
            vp = v_sb.bitcast(TR_DT)          # (C, NW) packed words
            idt = id_sb.bitcast(TR_DT)
            u = u_pool.tile([C, NCHUNK, C], FP16, tag="u_sb")
            for t in range(NT):
                t_ps = tr_ps_pool.tile([C, 512], TR_DT, tag="t_ps")
                for q in range(4):
                    k = 4 * t + q
                    nc.tensor.transpose(
                        t_ps[:, 128 * q : 128 * (q + 1)],
                        vp[:, 128 * k : 128 * (k + 1)],
                        idt,
                    )
                # de-interleave: contiguous fp16 read of the PSUM tile,
                # strided write into 8 contiguous V^T chunks (1x mode).
                src = t_ps.bitcast(FP16)      # (C, 1024): q(4) x c(128) x par(2)
                src3 = src.rearrange("p (q c r) -> p q r c", q=4, c=C)
                dst3 = u[:, 8 * t : 8 * t + 8, :].rearrange(
                    "p (q r) c -> p q r c", q=4
                )
                if t % 2 == 0:
                    nc.vector.tensor_copy(dst3, src3)
                else:
                    nc.scalar.copy(dst3, src3)
            umap[m] = u

        for m in range(LOOK):
            load_v(m)
        for m in range(TRLOOK):
            tr_v(m)

        for n in range(n_per_core + PIPE):
            if n < n_per_core:
                load_v(n + LOOK)
                v_sb = vmap.pop(n)
                u_sb = umap.pop(n)
                g_ps = g_ps_pool.tile([C, C], FP, tag="g_ps")

                def mm1(k0, k1):
                    for k in range(k0, k1):
                        ck = u_sb[:, k, :]
                        nc.tensor.matmul(
                            g_ps[:], ck, ck,
                            start=(k == 0), stop=(k == NCHUNK - 1),
                        )

                mm1(0, NCHUNK // 2)
                tr_v(n + TRLOOK)
                if len(pend) >= PIPE:
                    emit_stage2()
                mm1(NCHUNK // 2, NCHUNK)

                # softmax: rmin -> exp (with accumulated Z) -> fold gamma/Z
                # into numer -> transpose -> add I: abt = (gz*A)^T + I
                rmin = small_pool.tile([C, 1], FP, tag="rmin")
                nc.vector.tensor_reduce(rmin[:], g_ps[:], axis=AX.X, op=OP.min)
                numer = small_pool.tile([C, C], FP, tag="numer")
                zsum = small_pool.tile([C, 1], FP, tag="zsum")
                nc.scalar.activation(
                    numer[:], g_ps[:], AF.Exp,
                    bias=rmin[:], scale=-1.0, accum_out=zsum[:],
                )
                zinv = small_pool.tile([C, 1], FP, tag="zinv")
                nc.vector.reciprocal(zinv[:], zsum[:])
                gz = small_pool.tile([C, 1], FP, tag="gz")
                nc.vector.tensor_mul(gz[:], zinv[:], gam_sb[:])
                numer_s = small_pool.tile([C, C], FP, tag="numer_s")
                nc.vector.tensor_scalar(numer_s[:], numer[:], gz[:], None, OP.mult)
                pend.append((n, v_sb, numer_s))
            else:
                if pend:
                    emit_stage2()

    nc.compile()
    _CACHE[key] = nc
    return nc


def make_in_maps(x, gamma, n_per_core=N_PER_CORE):
    """Shard full inputs into per-core input maps (data-parallel over B*D)."""
    x = np.asarray(x, dtype=np.float32)
    gamma = np.asarray(gamma, dtype=np.float32).reshape(-1)
    gamma_b = np.full((C, 1), gamma[0], dtype=np.float32)
    ident = np.eye(C, dtype=np.float32)
    # v[n=(b,d)][c,s] = x[b,c,d,s] ; core i takes n in [i*npc, (i+1)*npc)
    xt = np.ascontiguousarray(
        x.reshape(B, C, D, S).transpose(0, 2, 1, 3)
    ).reshape(N_TOTAL, C, S).astype(np.float16)
    in_maps = []
    for i in range(N_CORES):
        xs = np.ascontiguousarray(xt[i * n_per_core : (i + 1) * n_per_core])
        in_maps.append({"xs": xs, "gamma_b": gamma_b, "ident": ident})
    return in_maps


def run_on_cores(x, gamma, trace=False, **kw):
    nc = build_program()
    in_maps = make_in_maps(x, gamma)
    res = run_bass_kernel_spmd(
        nc, in_maps, core_ids=list(range(N_CORES)), trace=trace, **kw
    )
    return res


def assemble_output(results):
    parts = [results[i]["out"] for i in range(N_CORES)]
    full = np.concatenate(parts, axis=0).astype(np.float32)  # (B*D, C, S)
    # reference returns a raw reinterpret of contiguous (B, D, C, H, W)
    return full.reshape(B, C, D, H, W)


def kernel(x, gamma):
    res = run_on_cores(x, gamma, trace=False)
    return assemble_output(res.results)
